# revision 6
# baseline (speedup 1.0000x reference)
"""Trainium2 Bass kernel for nn_CubEclayr (cubical-complex Euler characteristic curve).

Math: every cell's filtration value equals the value of its argmax pixel, so
    ECC[k] = sum_p w_p * 1[x_p <= T_k],
with integer weight w_p = 1 - Hcnt - Vcnt + Scnt from local >= comparisons
(tie-breaking provably cannot change the sum: tied pixels share a value).
No cumsum needed: the per-threshold masked count IS the cumulative ECC.
Provable bound |w| <= 3 (any square assignment forces >=1 h-edge and >=1
v-edge assignment to the same pixel), so a pair of thresholds packs exactly
into one fp32 accumulator as lo + 4096*hi: |sum lo| <= 3*452 = 1356 < 2048,
|acc| <= 1356*4097 < 2^24 -- exact integer arithmetic.

Sharding: batch-parallel, 2 batches (6 planes of 224x224) per core, 8 cores.

Layout per core: one padded 2D tensor per quantity, [128 partitions, 2713]:
12 tiles of 226 cols (left pad | 224 | right pad) + 1 spare init column.
Tile order is plane-contiguous: plane k = tiles 2k (rows 0..127) and 2k+1
(rows 128..223 + 32 pad rows), i.e. cols [452k, 452k+452).  x pads = 2.0
(> all thresholds, so pad pixels never count regardless of their weight).
Row shifts are "fat" full-tensor partition-shift DMAs + tiny seam fixups.
"""

import sys
import numpy as np

for _p in ("/opt/trn_rl_repo", "/opt/trn_rl_repo/concourse"):
    if _p not in sys.path:
        sys.path.insert(0, _p)

import dataclasses

import concourse.bass as bass
import concourse.tile as tile
from concourse import mybir
from concourse.bass_utils import run_bass_kernel_spmd

# Exact float32 bits of jnp.linspace(0.0, 1.0, 32) (4 entries differ from
# np.linspace by 1 ulp).
_TSEQ_BITS = np.array([
    0, 1023680776, 1032069384, 1036398988, 1040457992, 1042622794, 1044787596,
    1046952398, 1048846600, 1049929001, 1051011402, 1052093803, 1053176204,
    1054258605, 1055341006, 1056423407, 1057235208, 1057776408, 1058317609,
    1058858810, 1059400010, 1059941210, 1060482411, 1061023612, 1061564812,
    1062106012, 1062647213, 1063188414, 1063729614, 1064270814, 1064812015,
    1065353216], dtype=np.uint32)
TSEQ = _TSEQ_BITS.view(np.float32)

B, C, H, W = 16, 3, 224, 224
N_CORES = 8
B_PER_CORE = B // N_CORES           # 2
PLANES = B_PER_CORE * C             # 6
TILE_W = 226                        # 1 pad + 224 + 1 pad
PLANE_W = 2 * TILE_W                # 452
N_TILES = 2 * PLANES                # 12, order: p0T0 p0T1 p1T0 p1T1 ...
FREE = N_TILES * TILE_W             # 2712
FREEX = FREE + 1                    # +1 init column for shifted full-FD reads
F32 = mybir.dt.float32
ALU = mybir.AluOpType
SENTINEL = 2.0
PACK_K = 4096.0                     # hi-field scale in the paired histogram op
N_PAIRS = 16


def _register_pair_count():
    """Custom DVE op: out = ((in0<=s0) + (in0<=s1)*imm2) * in1; accum_out=sum.
    One pass computes two thresholds' weighted counts, packed in one fp32."""
    from concourse import dve_ops
    from concourse.dve_ops import OPS, DveOp, _SUB_OPCODE_FOR_NAME, _CUSTOM_DVE_ROW_BASE
    from concourse.dve_spec import Spec, Src0, Src1, C0, C1, C2, lower
    from concourse.dve_uop import DveOpSpec
    from operator import add

    name = "PAIR_COUNT_ANT"
    if name in _SUB_OPCODE_FOR_NAME:
        return next(op for op in OPS if op.name == name)

    def _ref(in0, in1, s0, s1, imm2):
        b = (((in0 <= s0) + (in0 <= s1) * imm2) * in1).astype(np.float32)
        return b, b.reshape(b.shape[0], -1).sum(axis=-1, keepdims=True)

    spec = Spec(
        body=((Src0 <= C0) + (Src0 <= C1) * C2) * Src1,
        accum=add,
        reference=_ref,
    )
    row = _CUSTOM_DVE_ROW_BASE + len(OPS)
    shas = {}
    for ver in ("v3", "v4"):
        s = DveOpSpec(name=name, opcode=row, uops=lower(spec, ver=ver), rd1_en=True)
        shas[ver] = s.sha(ver)
    op = DveOp(name, spec, subdim=False, uops_sha=shas)
    OPS.append(op)
    _SUB_OPCODE_FOR_NAME[name] = row
    dve_ops.CUSTOM_DVE_SPECS[name] = spec
    return op


def _split_drain_waits(nc, max_waits=1):
    """walrus codegen caps sem waits per instruction at 1 for CTRL/DMA
    pseudo-ops.  Hoist excess waits onto same-engine NOPs placed just before
    (engine FIFO order makes them execute first)."""
    for f in nc.m.functions:
        for bb in f.blocks:
            new_list = []
            for ins in bb.instructions:
                si = ins.sync_info
                if si and len(si.on_wait) > max_waits:
                    waits = list(si.on_wait)
                    extra, keep = waits[:-max_waits], waits[-max_waits:]
                    for ci, w in enumerate(extra):
                        new_list.append(mybir.InstNoOp(
                            name=f"{ins.name}_w{ci}", engine=ins.engine,
                            sync_info=mybir.SyncInfo(on_wait=[w], on_update=[]),
                        ))
                    ins.sync_info = mybir.SyncInfo(
                        on_wait=keep, on_update=list(si.on_update))
                    new_list.append(ins)
                else:
                    new_list.append(ins)
            bb.instructions[:] = new_list


def build_program():
    nc = bass.Bass()
    x_d = nc.dram_tensor("x", [B_PER_CORE, C, H, W], F32, kind="ExternalInput")
    y_d = nc.dram_tensor("y", [128, PLANES * 32], F32, kind="ExternalOutput")

    with tile.TileContext(nc) as tc:
        with tc.tile_pool(name="main", bufs=1) as pool:
            xt = pool.tile([128, FREEX], F32)    # x values (padded layout)
            xs = pool.tile([128, FREEX], F32)    # x shifted up one row
            hm = pool.tile([128, FREEX], F32)    # max(x, x-right)
            hs = pool.tile([128, FREEX], F32)    # hm shifted up one row
            g = pool.tile([128, FREEX], F32)     # 1[x >= x-right]
            eb = pool.tile([128, FREEX], F32)    # 1[x >= x-below]
            tt = pool.tile([128, FREEX], F32)    # 1[hm >= hs]
            tu = pool.tile([128, FREEX], F32)    # tt shifted down one row
            ebd = pool.tile([128, FREEX], F32)   # eb shifted down one row
            dd = pool.tile([128, FREEX], F32)    # tt - tu
            gc = pool.tile([128, FREEX], F32)    # 1 - g
            ab = pool.tile([128, FREEX], F32)    # A then B term
            wt = pool.tile([128, FREEX], F32)    # weight accumulator
            scratch = pool.tile([128, PLANE_W], F32)
            acc = pool.tile([128, PLANES * 32], F32)

            tiles12 = lambda tensor: tensor[:, 0:FREE].rearrange(
                "p (t c) -> p t c", c=TILE_W)

            # ---- load x into padded layout; pads/sentinels = 2.0 ----
            nc.vector.memset(xt[:], SENTINEL)
            dst = tiles12(xt)
            # T0 halves = even tiles, T1 halves = odd tiles
            src_t0 = x_d[:, :, 0:128, :].rearrange("b c r w -> r (b c) w")
            nc.sync.dma_start(out=dst[:, 0:N_TILES:2, 1:225], in_=src_t0)
            src_t1 = x_d[:, :, 128:224, :].rearrange("b c r w -> r (b c) w")
            nc.scalar.dma_start(out=dst[0:96, 1:N_TILES:2, 1:225], in_=src_t1)

            # ---- xs = x shifted up one row ----
            # Fat full-tensor shift: correct everywhere except the T0 bottom
            # seam (row 127 needs T1 row 0); T1 tail rows come from x pad rows
            # (= 2.0) automatically.
            nc.sync.dma_start(out=xs[0:127, :], in_=xt[1:128, :])
            xsv = tiles12(xs)
            nc.scalar.dma_start(out=xsv[127:128, 0:N_TILES:2, :],
                                in_=dst[0:1, 1:N_TILES:2, :])
            # partition-127 T1 spans + extra col: copy the (always 2.0) pad
            # row of xt (T1 partitions 96.. are pad rows; DVE memset cannot
            # start at partition 127).
            nc.scalar.dma_start(out=xsv[127:128, 1:N_TILES:2, :],
                                in_=tiles12(xt)[127:128, 1:N_TILES:2, :])
            nc.scalar.dma_start(out=xs[127:128, FREE:FREEX],
                                in_=xt[127:128, FREE:FREEX])

            # ---- elementwise prep (full-FD ops on [128, 2712]) ----
            x0 = xt[:, 0:FREE]
            x1 = xt[:, 1:FREEX]
            s0 = xs[:, 0:FREE]
            s1 = xs[:, 1:FREEX]
            nc.vector.tensor_tensor(out=hm[:, 0:FREE], in0=x0, in1=x1, op=ALU.max)
            nc.vector.tensor_tensor(out=hs[:, 0:FREE], in0=s0, in1=s1, op=ALU.max)
            nc.vector.tensor_tensor(out=g[:, 0:FREE], in0=x0, in1=x1, op=ALU.is_ge)
            nc.vector.tensor_tensor(out=eb[:, 0:FREE], in0=x0, in1=s0, op=ALU.is_ge)
            nc.vector.tensor_tensor(out=tt[:, 0:FREE], in0=hm[:, 0:FREE],
                                    in1=hs[:, 0:FREE], op=ALU.is_ge)
            # init extra col of chains read by shifted views
            nc.vector.memset(g[:, FREE:FREEX], 1.0)
            nc.vector.memset(tt[:, FREE:FREEX], 0.0)

            # ---- row-down shifts (tu = tt(r-1), ebd = eb(r-1)); phantom=1 ----
            nc.vector.memset(tu[0:1, :], 1.0)
            nc.sync.dma_start(out=tu[1:128, :], in_=tt[0:127, :])
            nc.scalar.dma_start(out=tiles12(tu)[0:1, 1:N_TILES:2, :],
                                in_=tiles12(tt)[127:128, 0:N_TILES:2, :])
            nc.vector.memset(ebd[0:1, :], 1.0)
            nc.scalar.dma_start(out=ebd[1:128, :], in_=eb[0:127, :])
            nc.sync.dma_start(out=tiles12(ebd)[0:1, 1:N_TILES:2, :],
                              in_=tiles12(eb)[127:128, 0:N_TILES:2, :])

            # ---- weight w = 1 - Hcnt - Vcnt + Scnt ----
            nc.vector.tensor_tensor(out=dd[:, 0:FREE], in0=tt[:, 0:FREE],
                                    in1=tu[:, 0:FREE], op=ALU.subtract)
            nc.vector.memset(dd[:, FREE:FREEX], 0.0)
            nc.vector.tensor_scalar(out=gc[:, 0:FREE], in0=g[:, 0:FREE],
                                    scalar1=-1.0, scalar2=1.0,
                                    op0=ALU.mult, op1=ALU.add)
            nc.vector.memset(gc[:, FREE:FREEX], 0.0)
            # A = (dd + 1) * g
            nc.vector.scalar_tensor_tensor(
                out=ab[:, 0:FREE], in0=dd[:, 0:FREE], scalar=1.0,
                in1=g[:, 0:FREE], op0=ALU.add, op1=ALU.mult)
            # w = ebd - eb
            nc.vector.tensor_tensor(out=wt[:, 0:FREE], in0=ebd[:, 0:FREE],
                                    in1=eb[:, 0:FREE], op=ALU.subtract)
            # w += gL
            nc.vector.tensor_tensor(out=wt[:, 1:FREEX], in0=wt[:, 1:FREEX],
                                    in1=g[:, 0:FREE], op=ALU.add)
            # w -= g
            nc.vector.tensor_tensor(out=wt[:, 0:FREE], in0=wt[:, 0:FREE],
                                    in1=g[:, 0:FREE], op=ALU.subtract)
            # w += A
            nc.vector.tensor_tensor(out=wt[:, 0:FREE], in0=wt[:, 0:FREE],
                                    in1=ab[:, 0:FREE], op=ALU.add)
            # B = (dd_L + 1) * gc_L  (into ab)
            nc.vector.scalar_tensor_tensor(
                out=ab[:, 1:FREEX], in0=dd[:, 0:FREE], scalar=1.0,
                in1=gc[:, 0:FREE], op0=ALU.add, op1=ALU.mult)
            nc.vector.memset(ab[:, 0:1], 0.0)
            # w = (B - 1) + w
            nc.vector.scalar_tensor_tensor(
                out=wt[:, 0:FREE], in0=ab[:, 0:FREE], scalar=-1.0,
                in1=wt[:, 0:FREE], op0=ALU.add, op1=ALU.add)

            # ---- histogram: acc[p, plane*32+k] = sum_f w * 1[x <= T_k] ----
            for plane in range(PLANES):
                xv = xt[:, plane * PLANE_W:(plane + 1) * PLANE_W]
                wv = wt[:, plane * PLANE_W:(plane + 1) * PLANE_W]
                for k in range(32):
                    nc.vector.scalar_tensor_tensor(
                        out=scratch[:], in0=xv, scalar=float(TSEQ[k]), in1=wv,
                        op0=ALU.is_le, op1=ALU.mult,
                        accum_out=acc[:, plane * 32 + k: plane * 32 + k + 1])

            nc.sync.dma_start(out=y_d[:], in_=acc[:])

    _split_drain_waits(nc)
    return nc


_NC_CACHE = None


def _get_nc():
    global _NC_CACHE
    if _NC_CACHE is None:
        _NC_CACHE = build_program()
    return _NC_CACHE


def kernel(x: np.ndarray) -> np.ndarray:
    x = np.ascontiguousarray(x, dtype=np.float32)
    assert x.shape == (B, C, H, W)
    nc = _get_nc()
    in_maps = [{"x": x[i * B_PER_CORE:(i + 1) * B_PER_CORE]} for i in range(N_CORES)]
    res = run_bass_kernel_spmd(nc, in_maps, core_ids=list(range(N_CORES)))
    out = np.empty((B, C * 32), dtype=np.float32)
    for i in range(N_CORES):
        acc = np.asarray(res.results[i]["y"], dtype=np.float64)  # [128, 192]
        ecc = acc.sum(axis=0).reshape(PLANES, 32)
        out[i * B_PER_CORE:(i + 1) * B_PER_CORE] = (
            ecc.reshape(B_PER_CORE, C * 32).astype(np.float32))
    return out


if __name__ == "__main__":
    rng = np.random.default_rng(0)
    xtest = rng.random((B, C, H, W), dtype=np.float32)
    out = kernel(xtest)
    print("kernel output shape:", out.shape)
    print(out[0, :8])


# revision 7
# speedup vs baseline: 1.3863x; 1.3863x over previous
"""Trainium2 Bass kernel for nn_CubEclayr (cubical-complex Euler characteristic curve).

Math: every cell's filtration value equals the value of its argmax pixel, so
    ECC[k] = sum_p w_p * 1[x_p <= T_k],
with integer weight w_p = 1 - Hcnt - Vcnt + Scnt from local >= comparisons
(tie-breaking provably cannot change the sum: tied pixels share a value).
No cumsum needed: the per-threshold masked count IS the cumulative ECC.
Provable bound |w| <= 3 (any square assignment forces >=1 h-edge and >=1
v-edge assignment to the same pixel), so a pair of thresholds packs exactly
into one fp32 accumulator as lo + 4096*hi: |sum lo| <= 3*452 = 1356 < 2048,
|acc| <= 1356*4097 < 2^24 -- exact integer arithmetic.

Sharding: batch-parallel, 2 batches (6 planes of 224x224) per core, 8 cores.

Layout per core: one padded 2D tensor per quantity, [128 partitions, 2713]:
12 tiles of 226 cols (left pad | 224 | right pad) + 1 spare init column.
Tile order is plane-contiguous: plane k = tiles 2k (rows 0..127) and 2k+1
(rows 128..223 + 32 pad rows), i.e. cols [452k, 452k+452).  x pads = 2.0
(> all thresholds, so pad pixels never count regardless of their weight).
Row shifts are "fat" full-tensor partition-shift DMAs + tiny seam fixups.
"""

import sys
import numpy as np

for _p in ("/opt/trn_rl_repo", "/opt/trn_rl_repo/concourse"):
    if _p not in sys.path:
        sys.path.insert(0, _p)

import dataclasses

import concourse.bass as bass
import concourse.tile as tile
from concourse import mybir
from concourse.bass_utils import run_bass_kernel_spmd

# Exact float32 bits of jnp.linspace(0.0, 1.0, 32) (4 entries differ from
# np.linspace by 1 ulp).
_TSEQ_BITS = np.array([
    0, 1023680776, 1032069384, 1036398988, 1040457992, 1042622794, 1044787596,
    1046952398, 1048846600, 1049929001, 1051011402, 1052093803, 1053176204,
    1054258605, 1055341006, 1056423407, 1057235208, 1057776408, 1058317609,
    1058858810, 1059400010, 1059941210, 1060482411, 1061023612, 1061564812,
    1062106012, 1062647213, 1063188414, 1063729614, 1064270814, 1064812015,
    1065353216], dtype=np.uint32)
TSEQ = _TSEQ_BITS.view(np.float32)

B, C, H, W = 16, 3, 224, 224
N_CORES = 8
B_PER_CORE = B // N_CORES           # 2
PLANES = B_PER_CORE * C             # 6
TILE_W = 226                        # 1 pad + 224 + 1 pad
PLANE_W = 2 * TILE_W                # 452
N_TILES = 2 * PLANES                # 12, order: p0T0 p0T1 p1T0 p1T1 ...
FREE = N_TILES * TILE_W             # 2712
FREEX = FREE + 1                    # +1 init column for shifted full-FD reads
F32 = mybir.dt.float32
BF16 = mybir.dt.bfloat16
ALU = mybir.AluOpType
SENTINEL = 2.0
PACK_K = 4096.0                     # hi-field scale in the paired histogram op
N_PAIRS = 16


def _register_pair_count():
    """Custom DVE op: out = ((in0<=s0) + (in0<=s1)*imm2) * in1; accum_out=sum.
    One pass computes two thresholds' weighted counts, packed in one fp32."""
    from concourse import dve_ops
    from concourse.dve_ops import OPS, DveOp, _SUB_OPCODE_FOR_NAME, _CUSTOM_DVE_ROW_BASE
    from concourse.dve_spec import Spec, Src0, Src1, C0, C1, C2, lower
    from concourse.dve_uop import DveOpSpec
    from operator import add

    name = "PAIR_COUNT_ANT"
    if name in _SUB_OPCODE_FOR_NAME:
        return next(op for op in OPS if op.name == name)

    def _ref(in0, in1, s0, s1, imm2):
        b = (((in0 <= s0) + (in0 <= s1) * imm2) * in1).astype(np.float32)
        return b, b.reshape(b.shape[0], -1).sum(axis=-1, keepdims=True)

    spec = Spec(
        body=((Src0 <= C0) + (Src0 <= C1) * C2) * Src1,
        accum=add,
        reference=_ref,
    )
    row = _CUSTOM_DVE_ROW_BASE + len(OPS)
    shas = {}
    for ver in ("v3", "v4"):
        s = DveOpSpec(name=name, opcode=row, uops=lower(spec, ver=ver), rd1_en=True)
        shas[ver] = s.sha(ver)
    op = DveOp(name, spec, subdim=False, uops_sha=shas)
    OPS.append(op)
    _SUB_OPCODE_FOR_NAME[name] = row
    dve_ops.CUSTOM_DVE_SPECS[name] = spec
    return op


def _split_drain_waits(nc, max_waits=1):
    """walrus codegen caps sem waits per instruction at 1 for CTRL/DMA
    pseudo-ops.  Hoist excess waits onto same-engine NOPs placed just before
    (engine FIFO order makes them execute first)."""
    for f in nc.m.functions:
        for bb in f.blocks:
            new_list = []
            for ins in bb.instructions:
                si = ins.sync_info
                if si and len(si.on_wait) > max_waits:
                    waits = list(si.on_wait)
                    extra, keep = waits[:-max_waits], waits[-max_waits:]
                    for ci, w in enumerate(extra):
                        new_list.append(mybir.InstNoOp(
                            name=f"{ins.name}_w{ci}", engine=ins.engine,
                            sync_info=mybir.SyncInfo(on_wait=[w], on_update=[]),
                        ))
                    ins.sync_info = mybir.SyncInfo(
                        on_wait=keep, on_update=list(si.on_update))
                    new_list.append(ins)
                else:
                    new_list.append(ins)
            bb.instructions[:] = new_list


def build_program():
    nc = bass.Bass()
    x_d = nc.dram_tensor("x", [B_PER_CORE, C, H, W], F32, kind="ExternalInput")
    y_d = nc.dram_tensor("y", [128, PLANES * 32], F32, kind="ExternalOutput")
    # DRAM bounce buffers for the partition-shift of tt/eb (SBUF->SBUF DMAs
    # serialize on one HW ring; DRAM round-trips spread over all 16).
    tt_bounce = nc.dram_tensor("ttb", [128, FREE], BF16)
    eb_bounce = nc.dram_tensor("ebb", [128, FREE], BF16)

    MAGIC = float(np.float32(1.0 / 31.0))
    R2 = float(np.float32(2.0 ** 23))

    with tile.TileContext(nc) as tc:
        with tc.tile_pool(name="main", bufs=1) as pool:
            # f32 value tensors
            xt = pool.tile([128, FREEX], F32)    # x values (padded layout)
            xs = pool.tile([128, FREEX], F32)    # x shifted up one row
            hm = pool.tile([128, FREEX], F32)    # max(x, x-right)
            hs = pool.tile([128, FREEX], F32)    # hm shifted up one row
            uu = pool.tile([128, FREEX], F32)    # 31*x ; then T(r-1)
            rr = pool.tile([128, FREEX], F32)    # round(31*x) ; then T(r)
            # bf16 mask/weight tensors (all small integers -- bf16-exact)
            g = pool.tile([128, FREEX], BF16)    # 1[x >= x-right]
            eb = pool.tile([128, FREEX], BF16)   # 1[x >= x-below]
            tt = pool.tile([128, FREEX], BF16)   # 1[hm >= hs]
            tu = pool.tile([128, FREEX], BF16)   # tt shifted down one row
            ebd = pool.tile([128, FREEX], BF16)  # eb shifted down one row
            dd = pool.tile([128, FREEX], BF16)   # tt - tu
            gc = pool.tile([128, FREEX], BF16)   # 1 - g
            ab = pool.tile([128, FREEX], BF16)   # A then B term
            wt = pool.tile([128, FREEX], BF16)   # weight accumulator
            c1 = pool.tile([128, FREEX], BF16)   # 1[x > T(r-1)]
            c2 = pool.tile([128, FREEX], BF16)   # 1[x > T(r)]
            bb = pool.tile([128, FREEX], BF16)   # exact bin index (0..62)
            scratch = pool.tile([128, PLANE_W], BF16)
            acc = pool.tile([128, PLANES * 32], F32)

            tiles12 = lambda tensor: tensor[:, 0:FREE].rearrange(
                "p (t c) -> p t c", c=TILE_W)

            # ---- load x into padded layout; pads/sentinels = 2.0 ----
            nc.vector.memset(xt[:], SENTINEL)
            dst = tiles12(xt)
            # T0 halves = even tiles, T1 halves = odd tiles
            src_t0 = x_d[:, :, 0:128, :].rearrange("b c r w -> r (b c) w")
            nc.sync.dma_start(out=dst[:, 0:N_TILES:2, 1:225], in_=src_t0)
            src_t1 = x_d[:, :, 128:224, :].rearrange("b c r w -> r (b c) w")
            nc.scalar.dma_start(out=dst[0:96, 1:N_TILES:2, 1:225], in_=src_t1)

            # ---- xs = x shifted up one row: load straight from DRAM ----
            nc.vector.memset(xs[:], SENTINEL)
            xsv = tiles12(xs)
            srcs_t0 = x_d[:, :, 1:129, :].rearrange("b c r w -> r (b c) w")
            nc.sync.dma_start(out=xsv[:, 0:N_TILES:2, 1:225], in_=srcs_t0)
            srcs_t1 = x_d[:, :, 129:224, :].rearrange("b c r w -> r (b c) w")
            nc.scalar.dma_start(out=xsv[0:95, 1:N_TILES:2, 1:225], in_=srcs_t1)

            # ---- compares / maxes on raw f32 values ----
            x0 = xt[:, 0:FREE]
            x1 = xt[:, 1:FREEX]
            s0 = xs[:, 0:FREE]
            s1 = xs[:, 1:FREEX]
            nc.vector.tensor_tensor(out=hm[:, 0:FREE], in0=x0, in1=x1, op=ALU.max)
            nc.vector.tensor_tensor(out=hs[:, 0:FREE], in0=s0, in1=s1, op=ALU.max)
            nc.vector.tensor_tensor(out=g[:, 0:FREE], in0=x0, in1=x1, op=ALU.is_ge)
            nc.vector.tensor_tensor(out=eb[:, 0:FREE], in0=x0, in1=s0, op=ALU.is_ge)
            nc.vector.tensor_tensor(out=tt[:, 0:FREE], in0=hm[:, 0:FREE],
                                    in1=hs[:, 0:FREE], op=ALU.is_ge)
            nc.vector.memset(g[:, FREE:FREEX], 1.0)
            nc.vector.memset(tt[:, FREE:FREEX], 0.0)

            # ---- exact bins: b = (r-1) + 1[x>T(r-1)] + 1[x>T(r)],
            #      r = rne(31*x), T(m) = m * fl(1/31) (bit-exact vs TSEQ) ----
            nc.vector.tensor_scalar(out=uu[:, 0:FREE], in0=x0, scalar1=31.0,
                                    scalar2=None, op0=ALU.mult)
            nc.vector.tensor_scalar(out=rr[:, 0:FREE], in0=uu[:, 0:FREE],
                                    scalar1=R2, scalar2=None, op0=ALU.add)
            nc.vector.tensor_scalar(out=rr[:, 0:FREE], in0=rr[:, 0:FREE],
                                    scalar1=-R2, scalar2=None, op0=ALU.add)
            # T(r-1) into uu (uu dead), T(r) into hm (hm dead after tt)
            nc.vector.tensor_scalar(out=uu[:, 0:FREE], in0=rr[:, 0:FREE],
                                    scalar1=-1.0, scalar2=MAGIC,
                                    op0=ALU.add, op1=ALU.mult)
            nc.vector.tensor_scalar(out=hm[:, 0:FREE], in0=rr[:, 0:FREE],
                                    scalar1=MAGIC, scalar2=None, op0=ALU.mult)
            nc.vector.tensor_tensor(out=c1[:, 0:FREE], in0=x0,
                                    in1=uu[:, 0:FREE], op=ALU.is_gt)
            nc.vector.tensor_tensor(out=c2[:, 0:FREE], in0=x0,
                                    in1=hm[:, 0:FREE], op=ALU.is_gt)
            # bb = (r - 1) + c1 + c2   (bf16-exact: ints <= 62)
            nc.vector.tensor_scalar(out=bb[:, 0:FREE], in0=rr[:, 0:FREE],
                                    scalar1=-1.0, scalar2=None, op0=ALU.add)
            nc.vector.tensor_tensor(out=bb[:, 0:FREE], in0=bb[:, 0:FREE],
                                    in1=c1[:, 0:FREE], op=ALU.add)
            nc.vector.tensor_tensor(out=bb[:, 0:FREE], in0=bb[:, 0:FREE],
                                    in1=c2[:, 0:FREE], op=ALU.add)

            # ---- row-down shifts of tt/eb via DRAM bounce (phantom = 1) ----
            nc.sync.dma_start(out=tt_bounce[:], in_=tt[:, 0:FREE])
            nc.scalar.dma_start(out=eb_bounce[:], in_=eb[:, 0:FREE])
            nc.vector.memset(tu[0:1, :], 1.0)
            nc.sync.dma_start(out=tu[1:128, 0:FREE], in_=tt_bounce[0:127, :])
            tbv = tt_bounce.rearrange("p (t c) -> p t c", c=TILE_W)
            nc.scalar.dma_start(out=tiles12(tu)[0:1, 1:N_TILES:2, :],
                                in_=tbv[127:128, 0:N_TILES:2, :])
            nc.vector.memset(ebd[0:1, :], 1.0)
            nc.scalar.dma_start(out=ebd[1:128, 0:FREE], in_=eb_bounce[0:127, :])
            ebv = eb_bounce.rearrange("p (t c) -> p t c", c=TILE_W)
            nc.sync.dma_start(out=tiles12(ebd)[0:1, 1:N_TILES:2, :],
                              in_=ebv[127:128, 0:N_TILES:2, :])

            # ---- weight w = 1 - Hcnt - Vcnt + Scnt  (bf16, 2x mode) ----
            nc.vector.tensor_tensor(out=dd[:, 0:FREE], in0=tt[:, 0:FREE],
                                    in1=tu[:, 0:FREE], op=ALU.subtract)
            nc.vector.memset(dd[:, FREE:FREEX], 0.0)
            nc.vector.tensor_scalar(out=gc[:, 0:FREE], in0=g[:, 0:FREE],
                                    scalar1=-1.0, scalar2=1.0,
                                    op0=ALU.mult, op1=ALU.add)
            nc.vector.memset(gc[:, FREE:FREEX], 0.0)
            # A = (dd + 1) * g
            nc.vector.scalar_tensor_tensor(
                out=ab[:, 0:FREE], in0=dd[:, 0:FREE], scalar=1.0,
                in1=g[:, 0:FREE], op0=ALU.add, op1=ALU.mult)
            # w = ebd - eb
            nc.vector.tensor_tensor(out=wt[:, 0:FREE], in0=ebd[:, 0:FREE],
                                    in1=eb[:, 0:FREE], op=ALU.subtract)
            # w += gL
            nc.vector.tensor_tensor(out=wt[:, 1:FREEX], in0=wt[:, 1:FREEX],
                                    in1=g[:, 0:FREE], op=ALU.add)
            # w -= g
            nc.vector.tensor_tensor(out=wt[:, 0:FREE], in0=wt[:, 0:FREE],
                                    in1=g[:, 0:FREE], op=ALU.subtract)
            # w += A
            nc.vector.tensor_tensor(out=wt[:, 0:FREE], in0=wt[:, 0:FREE],
                                    in1=ab[:, 0:FREE], op=ALU.add)
            # B = (dd_L + 1) * gc_L  (into ab)
            nc.vector.scalar_tensor_tensor(
                out=ab[:, 1:FREEX], in0=dd[:, 0:FREE], scalar=1.0,
                in1=gc[:, 0:FREE], op0=ALU.add, op1=ALU.mult)
            nc.vector.memset(ab[:, 0:1], 0.0)
            # w = (B - 1) + w
            nc.vector.scalar_tensor_tensor(
                out=wt[:, 0:FREE], in0=ab[:, 0:FREE], scalar=-1.0,
                in1=wt[:, 0:FREE], op0=ALU.add, op1=ALU.add)

            # ---- histogram over integer bins (bf16 2x):
            #      acc[p, plane*32+k] = sum_f w * 1[b <= k] ----
            for plane in range(PLANES):
                bv = bb[:, plane * PLANE_W:(plane + 1) * PLANE_W]
                wv = wt[:, plane * PLANE_W:(plane + 1) * PLANE_W]
                for k in range(32):
                    nc.vector.scalar_tensor_tensor(
                        out=scratch[:], in0=bv, scalar=float(k), in1=wv,
                        op0=ALU.is_le, op1=ALU.mult,
                        accum_out=acc[:, plane * 32 + k: plane * 32 + k + 1])

            nc.sync.dma_start(out=y_d[:], in_=acc[:])

    _split_drain_waits(nc)
    return nc


_NC_CACHE = None


def _get_nc():
    global _NC_CACHE
    if _NC_CACHE is None:
        _NC_CACHE = build_program()
    return _NC_CACHE


def kernel(x: np.ndarray) -> np.ndarray:
    x = np.ascontiguousarray(x, dtype=np.float32)
    assert x.shape == (B, C, H, W)
    nc = _get_nc()
    in_maps = [{"x": x[i * B_PER_CORE:(i + 1) * B_PER_CORE]} for i in range(N_CORES)]
    res = run_bass_kernel_spmd(nc, in_maps, core_ids=list(range(N_CORES)))
    out = np.empty((B, C * 32), dtype=np.float32)
    for i in range(N_CORES):
        acc = np.asarray(res.results[i]["y"], dtype=np.float64)  # [128, 192]
        ecc = acc.sum(axis=0).reshape(PLANES, 32)
        out[i * B_PER_CORE:(i + 1) * B_PER_CORE] = (
            ecc.reshape(B_PER_CORE, C * 32).astype(np.float32))
    return out


if __name__ == "__main__":
    rng = np.random.default_rng(0)
    xtest = rng.random((B, C, H, W), dtype=np.float32)
    out = kernel(xtest)
    print("kernel output shape:", out.shape)
    print(out[0, :8])


# revision 8
# speedup vs baseline: 1.5023x; 1.0837x over previous
"""Trainium2 Bass kernel for nn_CubEclayr (cubical-complex Euler characteristic curve).

Math: every cell's filtration value equals the value of its argmax pixel, so
    ECC[k] = sum_p w_p * 1[x_p <= T_k],
with integer weight w_p = 1 - Hcnt - Vcnt + Scnt from local >= comparisons
(tie-breaking provably cannot change the sum: tied pixels share a value).
No cumsum needed: the per-threshold masked count IS the cumulative ECC.
Provable bound |w| <= 3 (any square assignment forces >=1 h-edge and >=1
v-edge assignment to the same pixel), so a pair of thresholds packs exactly
into one fp32 accumulator as lo + 4096*hi: |sum lo| <= 3*452 = 1356 < 2048,
|acc| <= 1356*4097 < 2^24 -- exact integer arithmetic.

Sharding: batch-parallel, 2 batches (6 planes of 224x224) per core, 8 cores.

Layout per core: one padded 2D tensor per quantity, [128 partitions, 2713]:
12 tiles of 226 cols (left pad | 224 | right pad) + 1 spare init column.
Tile order is plane-contiguous: plane k = tiles 2k (rows 0..127) and 2k+1
(rows 128..223 + 32 pad rows), i.e. cols [452k, 452k+452).  x pads = 2.0
(> all thresholds, so pad pixels never count regardless of their weight).
Row shifts are "fat" full-tensor partition-shift DMAs + tiny seam fixups.
"""

import sys
import numpy as np

for _p in ("/opt/trn_rl_repo", "/opt/trn_rl_repo/concourse"):
    if _p not in sys.path:
        sys.path.insert(0, _p)

import dataclasses

import concourse.bass as bass
import concourse.tile as tile
from concourse import mybir
from concourse.bass_utils import run_bass_kernel_spmd

# Exact float32 bits of jnp.linspace(0.0, 1.0, 32) (4 entries differ from
# np.linspace by 1 ulp).
_TSEQ_BITS = np.array([
    0, 1023680776, 1032069384, 1036398988, 1040457992, 1042622794, 1044787596,
    1046952398, 1048846600, 1049929001, 1051011402, 1052093803, 1053176204,
    1054258605, 1055341006, 1056423407, 1057235208, 1057776408, 1058317609,
    1058858810, 1059400010, 1059941210, 1060482411, 1061023612, 1061564812,
    1062106012, 1062647213, 1063188414, 1063729614, 1064270814, 1064812015,
    1065353216], dtype=np.uint32)
TSEQ = _TSEQ_BITS.view(np.float32)

B, C, H, W = 16, 3, 224, 224
N_CORES = 8
B_PER_CORE = B // N_CORES           # 2
PLANES = B_PER_CORE * C             # 6
TILE_W = 226                        # 1 pad + 224 + 1 pad
PLANE_W = 2 * TILE_W                # 452
N_TILES = 2 * PLANES                # 12, order: p0T0 p0T1 p1T0 p1T1 ...
FREE = N_TILES * TILE_W             # 2712
FREEX = FREE + 1                    # +1 init column for shifted full-FD reads
F32 = mybir.dt.float32
BF16 = mybir.dt.bfloat16
ALU = mybir.AluOpType
SENTINEL = 2.0
PP = 21                             # partitions per plane in hist layout
HIST_FD = 2390                      # ceil(50176 / 21); 21*2390 = 50190
N_PLANE_PARTS = PLANES * PP         # 126


def _register_pair_count():
    """Custom DVE op: out = ((in0<=s0) + (in0<=s1)*imm2) * in1; accum_out=sum.
    One pass computes two thresholds' weighted counts, packed in one fp32."""
    from concourse import dve_ops
    from concourse.dve_ops import OPS, DveOp, _SUB_OPCODE_FOR_NAME, _CUSTOM_DVE_ROW_BASE
    from concourse.dve_spec import Spec, Src0, Src1, C0, C1, C2, lower
    from concourse.dve_uop import DveOpSpec
    from operator import add

    name = "PAIR_COUNT_ANT"
    if name in _SUB_OPCODE_FOR_NAME:
        return next(op for op in OPS if op.name == name)

    def _ref(in0, in1, s0, s1, imm2):
        b = (((in0 <= s0) + (in0 <= s1) * imm2) * in1).astype(np.float32)
        return b, b.reshape(b.shape[0], -1).sum(axis=-1, keepdims=True)

    spec = Spec(
        body=((Src0 <= C0) + (Src0 <= C1) * C2) * Src1,
        accum=add,
        reference=_ref,
    )
    row = _CUSTOM_DVE_ROW_BASE + len(OPS)
    shas = {}
    for ver in ("v3", "v4"):
        s = DveOpSpec(name=name, opcode=row, uops=lower(spec, ver=ver), rd1_en=True)
        shas[ver] = s.sha(ver)
    op = DveOp(name, spec, subdim=False, uops_sha=shas)
    OPS.append(op)
    _SUB_OPCODE_FOR_NAME[name] = row
    dve_ops.CUSTOM_DVE_SPECS[name] = spec
    return op


def _split_drain_waits(nc, max_waits=1):
    """walrus codegen caps sem waits per instruction at 1 for CTRL/DMA
    pseudo-ops.  Hoist excess waits onto same-engine NOPs placed just before
    (engine FIFO order makes them execute first)."""
    for f in nc.m.functions:
        for bb in f.blocks:
            new_list = []
            for ins in bb.instructions:
                si = ins.sync_info
                if si and len(si.on_wait) > max_waits:
                    waits = list(si.on_wait)
                    extra, keep = waits[:-max_waits], waits[-max_waits:]
                    for ci, w in enumerate(extra):
                        new_list.append(mybir.InstNoOp(
                            name=f"{ins.name}_w{ci}", engine=ins.engine,
                            sync_info=mybir.SyncInfo(on_wait=[w], on_update=[]),
                        ))
                    ins.sync_info = mybir.SyncInfo(
                        on_wait=keep, on_update=list(si.on_update))
                    new_list.append(ins)
                else:
                    new_list.append(ins)
            bb.instructions[:] = new_list


def build_program():
    nc = bass.Bass()
    x_d = nc.dram_tensor("x", [B_PER_CORE, C, H, W], F32, kind="ExternalInput")
    y_d = nc.dram_tensor("y", [128, 32], F32, kind="ExternalOutput")
    # DRAM bounce buffers for the partition-shift of tt/eb (SBUF->SBUF DMAs
    # serialize on one HW ring; DRAM round-trips spread over all 16).
    tt_bounce = nc.dram_tensor("ttb", [128, FREE], BF16)
    eb_bounce = nc.dram_tensor("ebb", [128, FREE], BF16)
    bb_bounce = nc.dram_tensor("bbb", [PLANES, PP * HIST_FD], BF16)
    ww_bounce = nc.dram_tensor("wwb", [PLANES, PP * HIST_FD], BF16)

    MAGIC = float(np.float32(1.0 / 31.0))
    R2 = float(np.float32(2.0 ** 23))

    with tile.TileContext(nc) as tc:
        with tc.tile_pool(name="main", bufs=1) as pool:
            # f32 value tensors
            xt = pool.tile([128, FREEX], F32)    # x values (padded layout)
            xs = pool.tile([128, FREEX], F32)    # x shifted up one row
            hm = pool.tile([128, FREEX], F32)    # max(x, x-right)
            hs = pool.tile([128, FREEX], F32)    # hm shifted up one row
            uu = pool.tile([128, FREEX], F32)    # 31*x ; then T(r-1)
            rr = pool.tile([128, FREEX], F32)    # round(31*x) ; then T(r)
            # bf16 mask/weight tensors (all small integers -- bf16-exact)
            g = pool.tile([128, FREEX], BF16)    # 1[x >= x-right]
            eb = pool.tile([128, FREEX], BF16)   # 1[x >= x-below]
            tt = pool.tile([128, FREEX], BF16)   # 1[hm >= hs]
            tu = pool.tile([128, FREEX], BF16)   # tt shifted down one row
            ebd = pool.tile([128, FREEX], BF16)  # eb shifted down one row
            dd = pool.tile([128, FREEX], BF16)   # tt - tu
            gc = pool.tile([128, FREEX], BF16)   # 1 - g
            ab = pool.tile([128, FREEX], BF16)   # A then B term
            wt = pool.tile([128, FREEX], BF16)   # weight accumulator
            c1 = pool.tile([128, FREEX], BF16)   # 1[x > T(r-1)]
            c2 = pool.tile([128, FREEX], BF16)   # 1[x > T(r)]
            bb = pool.tile([128, FREEX], BF16)   # exact bin index (0..62)
            bbh = pool.tile([128, HIST_FD], BF16)
            wth = pool.tile([128, HIST_FD], BF16)
            scratch = pool.tile([128, HIST_FD], BF16)
            acc = pool.tile([128, 32], F32)

            tiles12 = lambda tensor: tensor[:, 0:FREE].rearrange(
                "p (t c) -> p t c", c=TILE_W)

            # ---- load x into padded layout; pads/sentinels = 2.0 ----
            nc.vector.memset(xt[:], SENTINEL)
            dst = tiles12(xt)
            # T0 halves = even tiles, T1 halves = odd tiles
            src_t0 = x_d[:, :, 0:128, :].rearrange("b c r w -> r (b c) w")
            nc.sync.dma_start(out=dst[:, 0:N_TILES:2, 1:225], in_=src_t0)
            src_t1 = x_d[:, :, 128:224, :].rearrange("b c r w -> r (b c) w")
            nc.scalar.dma_start(out=dst[0:96, 1:N_TILES:2, 1:225], in_=src_t1)

            # ---- xs = x shifted up one row: load straight from DRAM ----
            nc.vector.memset(xs[:], SENTINEL)
            xsv = tiles12(xs)
            srcs_t0 = x_d[:, :, 1:129, :].rearrange("b c r w -> r (b c) w")
            nc.sync.dma_start(out=xsv[:, 0:N_TILES:2, 1:225], in_=srcs_t0)
            srcs_t1 = x_d[:, :, 129:224, :].rearrange("b c r w -> r (b c) w")
            nc.scalar.dma_start(out=xsv[0:95, 1:N_TILES:2, 1:225], in_=srcs_t1)

            # ---- compares / maxes on raw f32 values ----
            x0 = xt[:, 0:FREE]
            x1 = xt[:, 1:FREEX]
            s0 = xs[:, 0:FREE]
            s1 = xs[:, 1:FREEX]
            nc.vector.tensor_tensor(out=hm[:, 0:FREE], in0=x0, in1=x1, op=ALU.max)
            nc.vector.tensor_tensor(out=hs[:, 0:FREE], in0=s0, in1=s1, op=ALU.max)
            nc.vector.tensor_tensor(out=g[:, 0:FREE], in0=x0, in1=x1, op=ALU.is_ge)
            nc.vector.tensor_tensor(out=eb[:, 0:FREE], in0=x0, in1=s0, op=ALU.is_ge)
            nc.vector.tensor_tensor(out=tt[:, 0:FREE], in0=hm[:, 0:FREE],
                                    in1=hs[:, 0:FREE], op=ALU.is_ge)
            nc.vector.memset(g[:, FREE:FREEX], 1.0)
            nc.vector.memset(tt[:, FREE:FREEX], 0.0)

            # ---- exact bins: b = (r-1) + 1[x>T(r-1)] + 1[x>T(r)],
            #      r = rne(31*x), T(m) = m * fl(1/31) (bit-exact vs TSEQ) ----
            nc.vector.tensor_scalar(out=uu[:, 0:FREE], in0=x0, scalar1=31.0,
                                    scalar2=None, op0=ALU.mult)
            nc.vector.tensor_scalar(out=rr[:, 0:FREE], in0=uu[:, 0:FREE],
                                    scalar1=R2, scalar2=None, op0=ALU.add)
            nc.vector.tensor_scalar(out=rr[:, 0:FREE], in0=rr[:, 0:FREE],
                                    scalar1=-R2, scalar2=None, op0=ALU.add)
            # T(r-1) into uu (uu dead), T(r) into hm (hm dead after tt)
            nc.vector.tensor_scalar(out=uu[:, 0:FREE], in0=rr[:, 0:FREE],
                                    scalar1=-1.0, scalar2=MAGIC,
                                    op0=ALU.add, op1=ALU.mult)
            nc.vector.tensor_scalar(out=hm[:, 0:FREE], in0=rr[:, 0:FREE],
                                    scalar1=MAGIC, scalar2=None, op0=ALU.mult)
            nc.vector.tensor_tensor(out=c1[:, 0:FREE], in0=x0,
                                    in1=uu[:, 0:FREE], op=ALU.is_gt)
            nc.vector.tensor_tensor(out=c2[:, 0:FREE], in0=x0,
                                    in1=hm[:, 0:FREE], op=ALU.is_gt)
            # bb = (r - 1) + c1 + c2   (bf16-exact: ints <= 62)
            nc.vector.tensor_scalar(out=bb[:, 0:FREE], in0=rr[:, 0:FREE],
                                    scalar1=-1.0, scalar2=None, op0=ALU.add)
            nc.vector.tensor_tensor(out=bb[:, 0:FREE], in0=bb[:, 0:FREE],
                                    in1=c1[:, 0:FREE], op=ALU.add)
            nc.vector.tensor_tensor(out=bb[:, 0:FREE], in0=bb[:, 0:FREE],
                                    in1=c2[:, 0:FREE], op=ALU.add)

            # ---- row-down shifts of tt/eb via DRAM bounce (phantom = 1) ----
            nc.sync.dma_start(out=tt_bounce[:], in_=tt[:, 0:FREE])
            nc.scalar.dma_start(out=eb_bounce[:], in_=eb[:, 0:FREE])
            nc.vector.memset(tu[0:1, :], 1.0)
            nc.sync.dma_start(out=tu[1:128, 0:FREE], in_=tt_bounce[0:127, :])
            tbv = tt_bounce.rearrange("p (t c) -> p t c", c=TILE_W)
            nc.scalar.dma_start(out=tiles12(tu)[0:1, 1:N_TILES:2, :],
                                in_=tbv[127:128, 0:N_TILES:2, :])
            nc.vector.memset(ebd[0:1, :], 1.0)
            nc.scalar.dma_start(out=ebd[1:128, 0:FREE], in_=eb_bounce[0:127, :])
            ebv = eb_bounce.rearrange("p (t c) -> p t c", c=TILE_W)
            nc.sync.dma_start(out=tiles12(ebd)[0:1, 1:N_TILES:2, :],
                              in_=ebv[127:128, 0:N_TILES:2, :])

            # ---- weight w = 1 - Hcnt - Vcnt + Scnt  (bf16, 2x mode) ----
            nc.vector.tensor_tensor(out=dd[:, 0:FREE], in0=tt[:, 0:FREE],
                                    in1=tu[:, 0:FREE], op=ALU.subtract)
            nc.vector.memset(dd[:, FREE:FREEX], 0.0)
            nc.vector.tensor_scalar(out=gc[:, 0:FREE], in0=g[:, 0:FREE],
                                    scalar1=-1.0, scalar2=1.0,
                                    op0=ALU.mult, op1=ALU.add)
            nc.vector.memset(gc[:, FREE:FREEX], 0.0)
            # A = (dd + 1) * g
            nc.vector.scalar_tensor_tensor(
                out=ab[:, 0:FREE], in0=dd[:, 0:FREE], scalar=1.0,
                in1=g[:, 0:FREE], op0=ALU.add, op1=ALU.mult)
            # w = ebd - eb
            nc.vector.tensor_tensor(out=wt[:, 0:FREE], in0=ebd[:, 0:FREE],
                                    in1=eb[:, 0:FREE], op=ALU.subtract)
            # w += gL
            nc.vector.tensor_tensor(out=wt[:, 1:FREEX], in0=wt[:, 1:FREEX],
                                    in1=g[:, 0:FREE], op=ALU.add)
            # w -= g
            nc.vector.tensor_tensor(out=wt[:, 0:FREE], in0=wt[:, 0:FREE],
                                    in1=g[:, 0:FREE], op=ALU.subtract)
            # w += A
            nc.vector.tensor_tensor(out=wt[:, 0:FREE], in0=wt[:, 0:FREE],
                                    in1=ab[:, 0:FREE], op=ALU.add)
            # B = (dd_L + 1) * gc_L  (into ab)
            nc.vector.scalar_tensor_tensor(
                out=ab[:, 1:FREEX], in0=dd[:, 0:FREE], scalar=1.0,
                in1=gc[:, 0:FREE], op0=ALU.add, op1=ALU.mult)
            nc.vector.memset(ab[:, 0:1], 0.0)
            # w = (B - 1) + w
            nc.vector.scalar_tensor_tensor(
                out=wt[:, 0:FREE], in0=ab[:, 0:FREE], scalar=-1.0,
                in1=wt[:, 0:FREE], op0=ALU.add, op1=ALU.add)

            # ---- repack bb/wt to the hist layout via DRAM bounce:
            # plane j -> partitions [21j, 21j+21), 2390 items each (14-item
            # tail per plane stays at the memset values b=63 / w=0).
            def pack_out(t_sb, dram):
                dv = dram.rearrange("j (q f) -> j q f", f=HIST_FD)
                # dims [r, plane, col] on both sides
                sv = tiles12(t_sb)
                d_flat = dram.rearrange("j f -> (j f)")
                # T0 rows: dst flat = 50190*j + r*224 + c
                d0 = bass.AP(tensor=d_flat.tensor, offset=d_flat.offset,
                             ap=[[224, 128], [PP * HIST_FD, PLANES], [1, 224]])
                nc.sync.dma_start(out=d0, in_=sv[:, 0:N_TILES:2, 1:225])
                # T1 rows: dst flat = 50190*j + (128+r)*224 + c
                d1 = bass.AP(tensor=d_flat.tensor, offset=d_flat.offset + 128 * 224,
                             ap=[[224, 96], [PP * HIST_FD, PLANES], [1, 224]])
                nc.scalar.dma_start(out=d1, in_=sv[0:96, 1:N_TILES:2, 1:225])

            pack_out(bb, bb_bounce)
            pack_out(wt, ww_bounce)

            nc.vector.memset(bbh[:], 63.0)
            nc.vector.memset(wth[:], 0.0)
            for j in range(PLANES):
                eng = nc.sync if j % 2 == 0 else nc.scalar
                eng.dma_start(
                    out=bbh[j * PP:(j + 1) * PP, :],
                    in_=bb_bounce[j].rearrange("(q f) -> q f", f=HIST_FD))
                eng2 = nc.scalar if j % 2 == 0 else nc.sync
                eng2.dma_start(
                    out=wth[j * PP:(j + 1) * PP, :],
                    in_=ww_bounce[j].rearrange("(q f) -> q f", f=HIST_FD))

            # ---- histogram: one STT per threshold over all planes ----
            for k in range(32):
                nc.vector.scalar_tensor_tensor(
                    out=scratch[:], in0=bbh[:], scalar=float(k), in1=wth[:],
                    op0=ALU.is_le, op1=ALU.mult,
                    accum_out=acc[:, k:k + 1])

            nc.sync.dma_start(out=y_d[:], in_=acc[:])

    _split_drain_waits(nc)
    return nc


_NC_CACHE = None


def _get_nc():
    global _NC_CACHE
    if _NC_CACHE is None:
        _NC_CACHE = build_program()
    return _NC_CACHE


def kernel(x: np.ndarray) -> np.ndarray:
    x = np.ascontiguousarray(x, dtype=np.float32)
    assert x.shape == (B, C, H, W)
    nc = _get_nc()
    in_maps = [{"x": x[i * B_PER_CORE:(i + 1) * B_PER_CORE]} for i in range(N_CORES)]
    res = run_bass_kernel_spmd(nc, in_maps, core_ids=list(range(N_CORES)))
    out = np.empty((B, C * 32), dtype=np.float32)
    for i in range(N_CORES):
        acc = np.asarray(res.results[i]["y"], dtype=np.float64)  # [128, 32]
        ecc = acc[:N_PLANE_PARTS].reshape(PLANES, PP, 32).sum(axis=1)
        out[i * B_PER_CORE:(i + 1) * B_PER_CORE] = (
            ecc.reshape(B_PER_CORE, C * 32).astype(np.float32))
    return out


if __name__ == "__main__":
    rng = np.random.default_rng(0)
    xtest = rng.random((B, C, H, W), dtype=np.float32)
    out = kernel(xtest)
    print("kernel output shape:", out.shape)
    print(out[0, :8])
